# revision 29
# baseline (speedup 1.0000x reference)
"""MinkowskiGlobalPooling (average=True) segment-mean kernel for 8 trn2 cores.

Full inputs in, full output out. Internally:
  - rows are sharded across 8 cores (500k rows each),
  - host packs a per-core contiguous stream: for each chunk j and SBUF
    partition p, [T rows x (64 feats + ones-col) | T idx values],
  - each core builds one-hot masks (mask[p,b] = (idx[p]==b)) on VectorE and
    accumulates per-batch sums+counts via fp32 matmuls into PSUM [B, C+1]
    (last column = counts thanks to the ones column),
  - host sums the 8 per-core partials and divides.
"""

import numpy as np


def _ensure_import_path():
    try:
        import concourse.bass  # noqa: F401
    except ImportError:
        import sys

        for p in ("/opt/trn_rl_repo", "/root/.axon_site/_ro/trn_rl_repo"):
            if p not in sys.path:
                sys.path.insert(0, p)


N_CORES = 8
B = 32  # batches
C = 64  # channels
CP1 = C + 1  # channels + ones column
CP2 = C + 2  # + idx column (per-row payload in the stream)
N_TOTAL = 4_000_000
N_CORE = N_TOTAL // N_CORES  # 500_000 real rows per core
P = 128  # SBUF partitions; partitions 125..127 carry padding (idx=-1)
PREAL = 125  # N_CORE = PREAL * R exactly
R = N_CORE // PREAL  # 4000 rows per partition
T = 200  # rows per chunk (per partition)
NCHUNK = R // T  # 20
TM = 50  # rows per mask-generation op


def build_program(p=P, r=R, t=T, tm=TM, fbufs=3, mbufs=3, col_groups=4):
    """Build the per-core Bass program. All cores run the identical program.

    col_groups: PE column-group packing factor (1 or 4). With 4, consecutive
    row-slots stream into different 32-col strips of the PE array and
    accumulate into different PSUM partition strips; host sums the strips.
    """
    _ensure_import_path()
    import concourse.mybir as mybir
    from concourse import bacc
    from concourse.tile import TileContext

    f32 = mybir.dt.float32
    nchunk = r // t
    assert r % t == 0 and t % tm == 0 and (r // t * t) % col_groups == 0

    nc = bacc.Bacc()
    # stream[j, p, :] = [t rows x 65 payload | t idx values]
    stream = nc.dram_tensor("stream", [nchunk, p, t * CP2], f32, kind="ExternalInput")
    iota = nc.dram_tensor("iota", [p, tm * B], f32, kind="ExternalInput")
    out = nc.dram_tensor("out", [col_groups * B, CP1], f32, kind="ExternalOutput")

    with TileContext(nc) as tc:
        with (
            tc.tile_pool(name="const", bufs=1) as cpool,
            tc.tile_pool(name="feats", bufs=fbufs) as fpool,
            tc.tile_pool(name="mask", bufs=mbufs) as mpool,
            tc.tile_pool(name="psum", bufs=1, space="PSUM") as ppool,
            tc.tile_pool(name="outp", bufs=1) as opool,
        ):
            iota_sb = cpool.tile([p, tm * B], f32)
            nc.sync.dma_start(out=iota_sb[:], in_=iota[:, :])

            psum = ppool.tile([col_groups * B, CP1], f32)
            n_mm = nchunk * t
            if col_groups > 1:
                # Zero-valued "start" matmuls, one per column-group strip.
                # All real matmuls then accumulate (start=False), making the
                # result independent of the has_written-clear granularity.
                zero_mk = cpool.tile([p, B], f32)
                nc.vector.memset(zero_mk[:], 0.0)
                for g in range(col_groups):
                    nc.tensor.matmul(
                        psum[g * B : (g + 1) * B, :],
                        lhsT=zero_mk[:],
                        rhs=iota_sb[:, :CP1],
                        start=True,
                        stop=False,
                        tile_position=(0, g * B),
                        skip_group_check=True,
                    )
            k = 0
            for j in range(nchunk):
                ft = fpool.tile([p, t * CP2], f32)
                dma_eng = nc.sync if j % 2 == 0 else nc.scalar
                dma_eng.dma_start(out=ft[:], in_=stream[j])
                for s in range(t // tm):
                    mk = mpool.tile([p, tm * B], f32)
                    nc.vector.tensor_tensor(
                        out=mk[:].rearrange("p (t b) -> p t b", b=B),
                        in0=ft[:, t * CP1 + s * tm : t * CP1 + (s + 1) * tm]
                        .unsqueeze(2)
                        .to_broadcast([p, tm, B]),
                        in1=iota_sb[:].rearrange("p (t b) -> p t b", b=B),
                        op=mybir.AluOpType.is_equal,
                    )
                    for ts_ in range(tm):
                        tt = s * tm + ts_
                        g = k % col_groups
                        nc.tensor.matmul(
                            psum[g * B : (g + 1) * B, :],
                            lhsT=mk[:, ts_ * B : (ts_ + 1) * B],
                            rhs=ft[:, tt * CP1 : (tt + 1) * CP1],
                            start=(col_groups == 1 and k == 0),
                            stop=(k >= n_mm - col_groups),
                            tile_position=(0, g * B) if col_groups > 1 else None,
                            skip_group_check=(col_groups > 1),
                        )
                        k += 1
            out_sb = opool.tile([col_groups * B, CP1], f32)
            nc.vector.tensor_copy(out=out_sb[:], in_=psum[:])
            nc.sync.dma_start(out=out[:, :], in_=out_sb[:])
    nc.finalize()
    return nc


def host_prep(feats, batch_idx):
    """Build per-core input maps (packed stream layout) from full inputs."""
    feats = np.asarray(feats, dtype=np.float32)
    bi = np.asarray(batch_idx)
    n, c = feats.shape
    assert n == N_TOTAL and c == C, (n, c)

    idxf = bi.astype(np.float32)
    iota_rep = np.tile(np.arange(B, dtype=np.float32), (P, TM))  # [P, TM*B]

    in_maps = []
    for m in range(N_CORES):
        sl = slice(m * N_CORE, (m + 1) * N_CORE)
        fv = feats[sl].reshape(PREAL, NCHUNK, T, C)
        iv = idxf[sl].reshape(PREAL, NCHUNK, T)
        stream = np.empty((NCHUNK, P, T * CP2), dtype=np.float32)
        s4 = stream[:, :, : T * CP1].reshape(NCHUNK, P, T, CP1)
        s4[:, :PREAL, :, :C] = fv.transpose(1, 0, 2, 3)
        s4[:, PREAL:, :, :C] = 0.0
        s4[..., C] = 1.0
        si = stream[:, :, T * CP1 :].reshape(NCHUNK, P, T)
        si[:, :PREAL] = iv.transpose(1, 0, 2)
        si[:, PREAL:] = -1.0  # padding rows match no batch -> zero mask
        in_maps.append({"stream": stream, "iota": iota_rep})
    return in_maps


_CACHED_NC = None


def get_program():
    global _CACHED_NC
    if _CACHED_NC is None:
        _CACHED_NC = build_program()
    return _CACHED_NC


def run_on_cores(in_maps, trace=False):
    _ensure_import_path()
    from concourse.bass_utils import run_bass_kernel_spmd

    nc = get_program()
    res = run_bass_kernel_spmd(nc, in_maps, list(range(N_CORES)), trace=trace)
    return res


def finalize(per_core_outs):
    acc = np.zeros((B, CP1), dtype=np.float64)
    for o in per_core_outs:
        o = np.asarray(o, dtype=np.float64)
        acc += o.reshape(-1, B, CP1).sum(axis=0)
    sums = acc[:, :C]
    counts = acc[:, C]
    pooled = sums / np.maximum(counts, 1.0)[:, None]
    return pooled.astype(np.float32)


def kernel(feats, batch_idx, num_batches):
    assert int(num_batches) == B
    in_maps = host_prep(feats, batch_idx)
    res = run_on_cores(in_maps)
    return finalize([r["out"] for r in res.results])
